# revision 1
# baseline (speedup 1.0000x reference)
"""BertCRF loss kernel for 8 trn2 NeuronCores.

Strategy
--------
Data-parallel over batch: each of the 8 cores gets 32 sequences.

Per core (BL=32 local sequences, L=512, H=768, K=64):

* emit = features @ W on the PE in float32r (full rate), features
  host-transposed to [H, N] with n = t*BL + b (t-major) so DMA is dense and
  the scan consumes contiguous time slices.

* CRF forward scan in exp-space with a calibrated global per-step shift c:
      p_t = (expT^T @ p_{t-1}) * exp(emit_t + b - c)
  so one small matmul (PE) + one elementwise multiply (DVE) per step.
  d_t = log p_t + (t+1) c.

* The scan is a serial latency chain (~0.5us/step).  Serial depth is halved
  by exploiting the exponential (Birkhoff) contraction of products of
  positive matrices to rank-1:
    chain A: exact forward, t = 1..271 (blocks 0..16)
    chain B: forward from the all-ones vector planted at t=M=240
             (t = 241..511, blocks 15..31)
  For t >= ~M+8 the map M_{240->t} is numerically rank-1, so the true
  s_t = sum_k p_t[k] equals B's s~_t times one per-sequence scalar rho.
  rho is calibrated on host from the overlap region t in [256, 271] where
  chain A gives exact s_t and chain B gives s~_t.  Host assembles:
  t <= 271 from A, t >= 272 as rho * s~_t.  (Calibration error ~2e-6.)

* log-partition extraction: p_t kept per 16-step block in SBUF; a
  ones-vector matmul per block computes s_t for all t in bulk.

* gold path score: sum_t emit[b,t,tags]*mask via a host-built one-hot
  multiplied on GPSIMD and tree-reduced; transition + bias score terms are
  computed on host directly from tags/transition/mask (tiny integer work).
"""

import numpy as np
import ml_dtypes
from contextlib import ExitStack

import concourse.bass as bass
import concourse.tile as tile
from concourse import bacc, mybir
from concourse import bass_utils

F32 = mybir.dt.float32
F32R = mybir.dt.float32r
BF16 = mybir.dt.bfloat16

B, L, H, K = 256, 512, 768, 64
NCORES = 8
BL = B // NCORES            # 32 sequences per core
N = BL * L                  # 16384 columns, n = t*BL + b
TPB = 16                    # timesteps per block
NBLK = L // TPB             # 32 blocks
BLKN = TPB * BL             # 512 columns per block
HC = H // 128               # 6 contraction chunks

NSEG = 4                    # time-parallel cascade chains
# Segment boundaries (multiples of 16).  Later chains start only once their
# first E block has been DMA'd (~5us per slot), so they get shorter segments
# to equalize chain finish times.
SEG_BNDS = [0, 128, 256, 384]
RHO_W0, RHO_W1 = 4, 13      # overlap cols used for rho calibration

_CACHE = {}


def build():
    key = "nc"
    if key in _CACHE:
        return _CACHE[key]
    nc = bacc.Bacc("TRN2", target_bir_lowering=False, debug=False)

    fT = nc.dram_tensor("fT", [H, N], F32, kind="ExternalInput").ap()
    ohm = nc.dram_tensor("ohm", [K, N], BF16, kind="ExternalInput").ap()
    wm = nc.dram_tensor("wm", [H, K], F32, kind="ExternalInput").ap()
    expt = nc.dram_tensor("expt", [K, K], F32, kind="ExternalInput").ap()
    bvec = nc.dram_tensor("bvec", [K, 1], F32, kind="ExternalInput").ap()
    sout = nc.dram_tensor("sout", [1, N], BF16, kind="ExternalOutput").ap()
    gout = nc.dram_tensor("gout", [1, BL], F32, kind="ExternalOutput").ap()
    # overlap-block bulk sums of chains 0..NSEG-2 (for rho cascade)
    sbx = nc.dram_tensor("sbx", [1, (NSEG - 1) * BLKN], F32, kind="ExternalOutput").ap()


    with tile.TileContext(nc) as tc, ExitStack() as ctx:
        singles = ctx.enter_context(tc.tile_pool(name="singles", bufs=1))
        ftp = ctx.enter_context(tc.tile_pool(name="ftp", bufs=6))
        ohmp = ctx.enter_context(tc.tile_pool(name="ohmp", bufs=4))
        etmp = ctx.enter_context(tc.tile_pool(name="etmp", bufs=3))
        gtmp = ctx.enter_context(tc.tile_pool(name="gtmp", bufs=3))
        epool = ctx.enter_context(tc.tile_pool(name="epool", bufs=10))
        php = ctx.enter_context(tc.tile_pool(name="php", bufs=3))
        eps = ctx.enter_context(tc.tile_pool(name="eps", bufs=2, space="PSUM"))
        ssps = ctx.enter_context(tc.tile_pool(name="ssps", bufs=2, space="PSUM"))
        cps = [ctx.enter_context(tc.tile_pool(name=f"cps{c}", bufs=1, space="PSUM"))
               for c in range(NSEG)]

        w_sb = singles.tile([128, HC, K], F32R)
        nc.sync.dma_start(w_sb[:], wm.rearrange("(c p) k -> p c k", p=128).bitcast(F32R))
        expt_sb = singles.tile([K, K], F32)
        nc.sync.dma_start(expt_sb[:], expt)
        bvec_sb = singles.tile([K, 1], F32)
        nc.sync.dma_start(bvec_sb[:], bvec)
        ones_sb = singles.tile([K, 1], F32)
        nc.vector.memset(ones_sb[:], 1.0)
        S_sb = singles.tile([1, N], BF16)
        Gpart = singles.tile([K, NBLK * BL], F32)
        gsb = singles.tile([1, BL], F32)
        sbx_sb = singles.tile([1, (NSEG - 1) * BLKN], F32)
        seeds = [None] * NSEG

        # boundary blocks (seeds of chains 1..) stay resident until the
        # previous chain reads them at its end; allocate from singles.
        LONG_BLOCKS = {SEG_BNDS[c] // TPB for c in range(1, NSEG)}

        Et = [None] * NBLK
        ph = [[None] * NBLK for _ in range(NSEG)]

        def emit_block(i):
            n0 = i * BLKN
            fts = []
            for h in range(HC):
                ft = ftp.tile([128, BLKN], F32R, name=f"ft{h}", tag=f"ft{h}")
                eng = nc.sync if h < 3 else nc.scalar
                eng.dma_start(ft[:], fT[128 * h:128 * (h + 1), n0:n0 + BLKN].bitcast(F32R))
                fts.append(ft)
            ohm_t = ohmp.tile([K, BLKN], BF16, name="ohm_t", tag="ohm_t")
            nc.scalar.dma_start(ohm_t[:], ohm[:, n0:n0 + BLKN])
            pse = eps.tile([K, BLKN], F32, name="pse", tag="pse")
            for h in range(HC):
                nc.tensor.matmul(pse[:], w_sb[:, h, :], fts[h][:],
                                 start=(h == 0), stop=(h == HC - 1))
            if i in LONG_BLOCKS:
                Ei = singles.tile([K, BLKN], BF16, name=f"E{i}")
            else:
                Ei = epool.tile([K, BLKN], BF16, name=f"E{i}", tag="E")
            Et[i] = Ei
            nc.scalar.activation(out=Ei[:], in_=pse[:],
                                 func=mybir.ActivationFunctionType.Exp,
                                 bias=bvec_sb[:], scale=1.0)
            etm = etmp.tile([K, BLKN], BF16, name="etm", tag="etm")
            nc.scalar.copy(etm[:], pse[:])
            g2 = gtmp.tile([K, BLKN], F32, name="g2", tag="g2")
            nc.gpsimd.tensor_mul(g2[:], etm[:], ohm_t[:])
            v = g2.rearrange("p (t b) -> p t b", b=BL)
            nc.gpsimd.tensor_add(v[:, 0:8], v[:, 0:8], v[:, 8:16])
            nc.gpsimd.tensor_add(v[:, 0:4], v[:, 0:4], v[:, 4:8])
            nc.gpsimd.tensor_add(v[:, 0:2], v[:, 0:2], v[:, 2:4])
            nc.gpsimd.tensor_add(Gpart[:, i * BL:(i + 1) * BL], v[:, 0, :], v[:, 1, :])

        emitted = set()

        # emit-pipeline ops get a moderate scheduler-priority boost so the
        # PE/ACT keep draining DMA buffers even while scan-chain ops stall on
        # their serial dependencies (keeps the feature DMA saturated).
        def need(blk):
            if blk not in emitted:
                emitted.add(blk)
                with tc.high_priority(offset=125):
                    emit_block(blk)

        def bulk_s(ph_tile, dst, dst_off):
            for q in range(4):
                ps = ssps.tile([1, 128], F32, name="ssq", tag="ssq")
                nc.tensor.matmul(ps[:], ones_sb[:], ph_tile[:, q * 128:(q + 1) * 128],
                                 start=True, stop=True)
                nc.scalar.copy(dst[0:1, dst_off + q * 128: dst_off + (q + 1) * 128], ps[:])

        def step(c, t):
            # chain c step t: p_t = (expT^T @ p_{t-1}) * E_t
            db, dc = divmod(t, TPB)
            if ph[c][db] is None:
                ph[c][db] = php.tile([K, BLKN], F32, name=f"ph{c}_{db}", tag=f"ph{c}")
            sb_, sc = divmod(t - 1, TPB)
            if t == SEG_BNDS[c] + 1 and c > 0:
                src = seeds[c][:, 0:BL]
            else:
                src = ph[c][sb_][:, sc * BL:(sc + 1) * BL]
            ps = cps[c].tile([K, BL], F32, name=f"cp{c}", tag=f"cp{c}")
            nc.tensor.matmul(ps[:], expt_sb[:], src, start=True, stop=True)
            nc.vector.tensor_mul(ph[c][db][:, dc * BL:(dc + 1) * BL], ps[:],
                                 Et[db][:, dc * BL:(dc + 1) * BL])

        # --- prologue: chain starts ---
        # chain c covers t in [SEG_BNDS[c]+1, SEG_BNDS[c+1] + 12] (the overlap
        # beyond the boundary only feeds rho calibration, cols 1..12 suffice)
        ends = [min((SEG_BNDS + [L])[c + 1] + RHO_W1, L - 1) for c in range(NSEG)]
        for c in range(NSEG):
            if c == 0:
                need(0)
                ph[0][0] = php.tile([K, BLKN], F32, name="ph0_0", tag="ph0")
                nc.vector.tensor_copy(ph[0][0][:, 0:BL], Et[0][:, 0:BL])
            else:
                seeds[c] = singles.tile([K, BL], F32, name=f"seed{c}")
                nc.vector.memset(seeds[c][:], 1.0)

        # --- concurrent chains, fed round-robin ---
        maxsteps = max(ends[c] - SEG_BNDS[c] for c in range(NSEG))
        # reversed order: chain 3 starts latest (its first feature block is
        # the 4th DMA delivery), so it gets the first slot of every round
        for rr in range(maxsteps):
            for c in reversed(range(NSEG)):
                t = SEG_BNDS[c] + 1 + rr
                if t > ends[c]:
                    continue
                need(t // TPB)
                step(c, t)
                if t % TPB == TPB - 1 or t == ends[c]:
                    j = t // TPB
                    if c < NSEG - 1 and j == SEG_BNDS[c + 1] // TPB:
                        # overlap block: goes to the rho side-channel
                        bulk_s(ph[c][j], sbx_sb, c * BLKN)
                    else:
                        bulk_s(ph[c][j], S_sb, j * BLKN)

        # --- gold finale: G64[k,b] = sum_blk Gpart, then sum_k via ones-MM ---
        g64 = singles.tile([K, BL], F32)
        nc.vector.tensor_reduce(g64[:], Gpart.rearrange("p (c b) -> p b c", b=BL),
                                axis=mybir.AxisListType.X, op=mybir.AluOpType.add)
        psg = ssps.tile([1, BL], F32, name="psg", tag="ssq")
        nc.tensor.matmul(psg[:], ones_sb[:], g64[:], start=True, stop=True)
        nc.scalar.copy(gsb[:], psg[:])
        nc.sync.dma_start(gout, gsb[:])
        nc.sync.dma_start(sbx, sbx_sb[:])
        nc.sync.dma_start(sout, S_sb[:])

    nc.compile()
    _CACHE[key] = nc
    return nc


def prepare(features, W, b, transition, tags, mask):
    """Host-side prep: per-core input maps + host-only scalars."""
    features = np.asarray(features)
    W = np.asarray(W, dtype=np.float32)
    b = np.asarray(b, dtype=np.float32)
    transition = np.asarray(transition, dtype=np.float32)
    tags = np.asarray(tags)
    mask = np.asarray(mask)

    # Per-step growth constant c: the scan tracks p_t = exp(d_t - (t+1)c), so
    # c must match the true mean log-growth of the forward recursion or p
    # drifts towards f32 overflow/underflow over 512 steps.  Growth is set by
    # the Perron eigenvalue of exp(T) modulated by the mean emission factor
    # E[e^emit_k] = e^{var_k/2}; a 64-dim host power iteration nails it.
    expT64 = np.exp(transition.astype(np.float64))
    evar = (W.astype(np.float64) ** 2).sum(0)
    emod = np.exp(evar / 2.0)
    v = np.ones(K, dtype=np.float64)
    c_acc = 0.0
    for it in range(60):
        v = (expT64.T @ v) * emod
        g = v.sum()
        if it >= 30:
            c_acc += np.log(g)
        v /= g
    c = float(c_acc / 30.0)

    expT = np.exp(transition).astype(np.float32)
    bvec = (b - c).astype(np.float32)[:, None]
    wm = np.ascontiguousarray(W, dtype=np.float32)

    in_maps = []
    for ci in range(NCORES):
        b0 = ci * BL
        fsh = features[b0:b0 + BL]                       # [BL, L, H]
        fTl = np.ascontiguousarray(fsh.transpose(2, 1, 0).reshape(H, N),
                                   dtype=np.float32)     # [H, t*BL+b]
        tsh = tags[b0:b0 + BL].astype(np.int64)          # [BL, L]
        msh = mask[b0:b0 + BL]                           # [BL, L]
        oh = (np.arange(K, dtype=np.int64)[:, None, None] == tsh.T[None, :, :])
        oh = (oh & msh.T[None, :, :]).reshape(K, N).astype(np.float32)
        in_maps.append({"fT": fTl,
                        "ohm": np.ascontiguousarray(oh.astype(ml_dtypes.bfloat16)),
                        "wm": wm, "expt": expT, "bvec": bvec})

    lens = mask.sum(1).astype(np.int64)
    # host part of gold score: transitions + bias gathers (tiny integer work)
    maskf = mask.astype(np.float64)
    trans_sc = transition.astype(np.float64)[tags[:, :-1], tags[:, 1:]]  # [B, L-1]
    host_score = (trans_sc * maskf[:, 1:]).sum(1)
    host_score += (b.astype(np.float64)[tags] * maskf).sum(1)
    return in_maps, lens, c, host_score


def finish(results, lens, c, host_score):
    """Assemble outputs: cascade-calibrate chains 1..NSEG-1, pick s at len-1."""
    out = np.empty(B, dtype=np.float32)
    for ci in range(NCORES):
        S = results[ci]["sout"][0].astype(np.float64)    # [N], n = t*BL+b
        G = results[ci]["gout"][0].astype(np.float64)    # [BL]
        SX = results[ci]["sbx"][0].astype(np.float64)    # [(NSEG-1)*BLKN]
        # logr[c] = log rho of chain c (0 for chain 0), cascaded through the
        # overlap blocks: chain c-1's overlap bulk (SX, already calibrated by
        # logr[c-1]) vs chain c's own values of the same t (in S).
        logr = np.zeros((NSEG, BL))
        for cseg in range(1, NSEG):
            t0 = SEG_BNDS[cseg]
            acc = np.zeros(BL)
            cnt = 0
            for w in range(RHO_W0, RHO_W1):
                t = t0 + w
                s_prev = SX[(cseg - 1) * BLKN + w * BL:(cseg - 1) * BLKN + (w + 1) * BL]
                s_cur = S[t * BL:(t + 1) * BL]
                acc += np.log(s_prev) - np.log(s_cur)
                cnt += 1
            logr[cseg] = logr[cseg - 1] + acc / cnt
        for bl in range(BL):
            bg = ci * BL + bl
            ln = int(lens[bg])
            t = ln - 1
            cseg = max(i for i in range(NSEG) if SEG_BNDS[i] <= t)
            # first RHO_W1 cols of a segment boundary block: use the previous
            # chain's overlap values (the overlap chain stops at col RHO_W1-1;
            # later cols come from chain cseg itself, already rank-1-converged)
            if cseg > 0 and t < SEG_BNDS[cseg] + RHO_W1:
                log_s = np.log(SX[(cseg - 1) * BLKN + (t - SEG_BNDS[cseg]) * BL + bl]) \
                    + logr[cseg - 1][bl]
            else:
                log_s = np.log(S[t * BL + bl]) + logr[cseg][bl]
            log_z = log_s + ln * c
            out[bg] = log_z - (G[bl] + host_score[bg])
    return out


def kernel(features, W, b, transition, tags, mask):
    nc = build()
    in_maps, lens, c, host_score = prepare(features, W, b, transition, tags, mask)
    res = bass_utils.run_bass_kernel_spmd(nc, in_maps, core_ids=list(range(NCORES)))
    return finish(res.results, lens, c, host_score)



# revision 2
# speedup vs baseline: 3.4942x; 3.4942x over previous
"""BertCRF loss kernel for 8 trn2 NeuronCores.

Strategy
--------
Data-parallel over batch: each of the 8 cores gets BL=32 sequences.

Per core:

* Column layout n = rr*512 + c*32 + b with t = 32*c + rr: 16 cascade
  chains (one per 32-step segment of the sequence) advance in lockstep
  "rounds"; round rr touches one dense 512-column stripe of every tensor.

* emit = features @ W on the PE in fp8-e4m3 DoubleRow mode (2 k-tiles of
  128 per instruction, 3 instructions per 512-column block).  W is
  host-scaled by 32 and padded with a zero 65th output column so that
  E = exp(emit/32 + b - c_shift) carries a built-in row of ones
  (row 64), used to propagate path sums through the scan for free.

* CRF forward scan in exp-space with a calibrated per-step shift c_shift:
      p_t = (expT^T p_{t-1}) * E_t
  One [64,65]-bf16 matmul (PE) + one elementwise multiply (DVE) per round
  per stream; 2 streams of 256 columns.  expT is extended with a 65th
  ones-column, so PSUM row 64 of round rr holds the element sums s of the
  round rr-1 states; the multiply copies it into the bf16 state tile's
  row 64 (E row 64 == 1).  All s values ship to host in one DMA at the end.

* Chains 1..15 are seeded with ones planted one step before their segment;
  chain 0 is seeded with x = solve(expT^T, 1) so its first state is exactly
  E_0.  After the 32 main rounds, 8 overlap (tail) rounds extend each chain
  into the next segment; the host calibrates per-chain scale factors rho
  from the overlap (rank-1 convergence of products of positive matrices)
  and cascades them.

* gold path score (emissions, transitions, bias) is computed on host in
  fp32/64 directly from features/tags/mask (exact, no device traffic).
"""

import numpy as np
import ml_dtypes
from contextlib import ExitStack

import concourse.bass as bass
import concourse.tile as tile
from concourse import bacc, mybir
from concourse import bass_utils

F32 = mybir.dt.float32
BF16 = mybir.dt.bfloat16
F8 = mybir.dt.float8e4
E4M3 = ml_dtypes.float8_e4m3
BF16NP = ml_dtypes.bfloat16

B, L, H, K = 256, 512, 768, 64
NCORES = 8
BL = B // NCORES            # 32 sequences per core
N = BL * L                  # 16384 columns
NSEG = 16                   # cascade chains
SEG = L // NSEG             # 32 main rounds
OV = 8                      # overlap (tail) rounds
RTOT = SEG + OV             # 40 rounds of state
CONV = 7                    # below this in-segment offset, use prev chain's value
CAL_LO, CAL_HI = 3, 7       # overlap offsets used for rho calibration
WSCALE = 32.0               # host scale on W to centre fp8 range
RW = 512                    # columns per round (NSEG * BL)
HC = 6                      # 128-contraction chunks in H

_CACHE = {}


def build():
    key = "nc"
    if key in _CACHE:
        return _CACHE[key]
    nc = bacc.Bacc("TRN2", target_bir_lowering=False, debug=False)

    ft = nc.dram_tensor("ft", [128, HC, N], F8, kind="ExternalInput").ap()
    wq = nc.dram_tensor("wq", [128, HC, 80], F8, kind="ExternalInput").ap()
    expt = nc.dram_tensor("expt", [K, K + 1], BF16, kind="ExternalInput").ap()
    bvec = nc.dram_tensor("bvec", [K + 1, 1], F32, kind="ExternalInput").ap()
    seed = nc.dram_tensor("seed", [K, RW], BF16, kind="ExternalInput").ap()
    sout = nc.dram_tensor("sout", [1, RTOT * RW], BF16, kind="ExternalOutput").ap()

    with tile.TileContext(nc) as tc, ExitStack() as ctx:
        singles = ctx.enter_context(tc.tile_pool(name="singles", bufs=1))
        ftp = ctx.enter_context(tc.tile_pool(name="ftp", bufs=3))
        eps = ctx.enter_context(tc.tile_pool(name="eps", bufs=2, space="PSUM"))
        cpsA = ctx.enter_context(tc.tile_pool(name="cpsA", bufs=1, space="PSUM"))
        cpsB = ctx.enter_context(tc.tile_pool(name="cpsB", bufs=1, space="PSUM"))
        cpsC = ctx.enter_context(tc.tile_pool(name="cpsC", bufs=1, space="PSUM"))

        wq_sb = singles.tile([128, HC, 80], F8)
        nc.sync.dma_start(wq_sb[:], wq)
        expt_sb = singles.tile([K, K + 1], BF16)
        nc.sync.dma_start(expt_sb[:], expt)
        bvec_sb = singles.tile([K + 1, 1], F32)
        nc.sync.dma_start(bvec_sb[:], bvec)
        seed_sb = singles.tile([K, RW], BF16)
        nc.sync.dma_start(seed_sb[:], seed)

        E_ALL = singles.tile([K + 1, SEG * RW], BF16)
        PH = singles.tile([K + 1, RTOT * RW], BF16)

        def emit_block(blk, ft_t, s):
            # one 512-column stripe: 3 DoubleRow matmuls + exp
            pse = eps.tile([80, RW], F32, name="pse", tag="pse")
            for i in range(3):
                nc.tensor.matmul(pse[:], wq_sb[:, 2 * i:2 * i + 2, :],
                                 ft_t[:, 2 * i:2 * i + 2, s * RW:(s + 1) * RW],
                                 start=(i == 0), stop=(i == 2),
                                 perf_mode=mybir.MatmulPerfMode.DoubleRow)
            nc.scalar.activation(out=E_ALL[:, blk * RW:(blk + 1) * RW],
                                 in_=pse[0:K + 1, :],
                                 func=mybir.ActivationFunctionType.Exp,
                                 bias=bvec_sb[:], scale=1.0 / WSCALE)

        def scan_round(rr):
            # two streams of 256 columns; src = previous round's state
            for st, pool in ((0, cpsA), (1, cpsB)):
                lo = st * 256
                if rr == 0:
                    src = seed_sb[:, lo:lo + 256]
                else:
                    src = PH[0:K, (rr - 1) * RW + lo:(rr - 1) * RW + lo + 256]
                ps = pool.tile([K + 1, 256], F32, name=f"cp{st}", tag=f"cp{st}")
                nc.tensor.matmul(ps[:], expt_sb[:], src, start=True, stop=True)
                nc.vector.tensor_mul(PH[:, rr * RW + lo:rr * RW + lo + 256],
                                     ps[:], E_ALL[:, rr * RW + lo:rr * RW + lo + 256])

        def tail_round(j):
            # chains 0..14 step into the next segment; E slice shifts one slot
            rr = SEG + j
            for st, pool, lo, w in ((0, cpsA, 0, 256), (1, cpsB, 256, 224)):
                src = PH[0:K, (rr - 1) * RW + lo:(rr - 1) * RW + lo + w]
                ps = pool.tile([K + 1, w], F32, name=f"tp{st}", tag=f"cp{st}")
                nc.tensor.matmul(ps[:], expt_sb[:], src, start=True, stop=True)
                nc.vector.tensor_mul(PH[:, rr * RW + lo:rr * RW + lo + w],
                                     ps[:], E_ALL[:, j * RW + 32 + lo:j * RW + 32 + lo + w])
            if j == 0:
                # chain 15's t=511 sums: matmul only for PSUM row 64; the
                # multiply uses an arbitrary E chunk (row 64 is ones).
                src = PH[0:K, (rr - 1) * RW + 480:(rr - 1) * RW + 512]
                ps = cpsC.tile([K + 1, 32], F32, name="tpc", tag="cpc")
                nc.tensor.matmul(ps[:], expt_sb[:], src, start=True, stop=True)
                nc.vector.tensor_mul(PH[:, rr * RW + 480:rr * RW + 512],
                                     ps[:], E_ALL[:, 0:32])

        for dblk in range(SEG // 2):
            ft_t = ftp.tile([128, HC, 2 * RW], F8, name="ft_t", tag="ft_t")
            nc.sync.dma_start(ft_t[:], ft[:, :, dblk * 2 * RW:(dblk + 1) * 2 * RW])
            for s in range(2):
                blk = 2 * dblk + s
                emit_block(blk, ft_t, s)
                scan_round(blk)
        for j in range(OV):
            tail_round(j)

        nc.sync.dma_start(sout, PH[K:K + 1, :])

    nc.compile()
    _CACHE[key] = nc
    return nc


def prepare(features, W, b, transition, tags, mask):
    """Host-side prep: per-core inputs + host-only scalars."""
    features = np.asarray(features, dtype=np.float32)
    W = np.asarray(W, dtype=np.float32)
    b = np.asarray(b, dtype=np.float32)
    transition = np.asarray(transition, dtype=np.float32)
    tags = np.asarray(tags).astype(np.int64)
    mask = np.asarray(mask).astype(bool)

    # Per-step growth constant c_shift: Perron eigenvalue of exp(T) modulated
    # by the mean emission factor; host power iteration (as in the reference
    # exp-space formulation) keeps p from drifting out of range.
    expT64 = np.exp(transition.astype(np.float64))
    evar = (W.astype(np.float64) ** 2).sum(0)
    emod = np.exp(evar / 2.0)
    v = np.ones(K, dtype=np.float64)
    c_acc = 0.0
    for it in range(60):
        v = (expT64.T @ v) * emod
        g = v.sum()
        if it >= 30:
            c_acc += np.log(g)
        v /= g
    c_shift = float(c_acc / 30.0)

    expT = np.zeros((K, K + 1), dtype=np.float64)
    expT[:, :K] = expT64
    expT[:, K] = 1.0
    expT16 = expT.astype(BF16NP)

    bvec = np.zeros((K + 1, 1), dtype=np.float32)
    bvec[:K, 0] = b - c_shift

    wqf = np.zeros((128, HC, 80), dtype=np.float32)
    wqf[:, :, :K] = (W * WSCALE).reshape(HC, 128, K).transpose(1, 0, 2)
    wq8 = wqf.astype(E4M3)

    # seeds: chain 0 gets x with expT^T x = 1 (exact first state); others ones
    x0 = np.linalg.solve(expT64.T, np.ones(K))
    seedf = np.ones((K, RW), dtype=np.float64)
    for bcol in range(BL):
        seedf[:, bcol] = x0
    seed16 = seedf.astype(BF16NP)

    in_maps = []
    for ci in range(NCORES):
        b0 = ci * BL
        fsh = features[b0:b0 + BL]                      # [BL, L, H]
        # n = rr*512 + c*32 + b ; t = 32c + rr ; h = ch*128 + p
        arr = fsh.transpose(2, 1, 0)                    # [H, L, BL]
        arr = arr.reshape(HC, 128, NSEG, SEG, BL)       # [ch, p, c, rr, b]
        arr = arr.transpose(1, 0, 3, 2, 4)              # [p, ch, rr, c, b]
        ft8 = np.ascontiguousarray(arr).reshape(128, HC, N).astype(E4M3)
        in_maps.append({"ft": ft8, "wq": wq8, "expt": expT16,
                        "bvec": bvec, "seed": seed16})

    lens = mask.sum(1).astype(np.int64)

    # full gold path score on host (exact): emissions + transitions + bias
    maskf = mask.astype(np.float64)
    trans_sc = transition.astype(np.float64)[tags[:, :-1], tags[:, 1:]]
    gold = (trans_sc * maskf[:, 1:]).sum(1)
    gold += (b.astype(np.float64)[tags] * maskf).sum(1)
    Wt = W.T.astype(np.float32)                        # [K, H]
    for b0 in range(0, B, 32):
        Wg = Wt[tags[b0:b0 + 32]]                      # [32, L, H]
        ge = np.einsum('blh,blh->bl', features[b0:b0 + 32], Wg,
                       optimize=True).astype(np.float64)
        gold[b0:b0 + 32] += (ge * maskf[b0:b0 + 32]).sum(1)

    return in_maps, lens, c_shift, gold


def finish(results, lens, c_shift, gold):
    """Assemble outputs: cascade-calibrate chains, pick s at len-1."""
    out = np.empty(B, dtype=np.float32)
    for ci in range(NCORES):
        S = results[ci]["sout"][0].astype(np.float64).reshape(RTOT, NSEG, BL)
        logS = np.log(S)
        # logS[rr, c, b] = log sum of chain c's state *entering* round rr:
        #   rr in [1, 32]: chain c main state at t = 32c + rr - 1
        #   rr in [33, 39]: chain c overlap state at t = 32(c+1) + rr - 33
        logr = np.zeros((NSEG, BL))
        for c in range(1, NSEG):
            acc = np.zeros(BL)
            for w in range(CAL_LO, CAL_HI):
                s_prev = logS[33 + w, c - 1]   # chain c-1 overlap at t=32c+w
                s_cur = logS[w + 1, c]         # chain c main at t=32c+w
                acc += s_prev - s_cur
            logr[c] = logr[c - 1] + acc / (CAL_HI - CAL_LO)
        for bl in range(BL):
            bg = ci * BL + bl
            ln = int(lens[bg])
            t = ln - 1
            c = t // SEG
            w = t % SEG
            if c > 0 and w < CONV:
                log_s = logS[33 + w, c - 1, bl] + logr[c - 1, bl]
            else:
                log_s = logS[w + 1, c, bl] + logr[c, bl]
            log_z = log_s + ln * c_shift
            out[bg] = log_z - gold[bg]
    return out


def kernel(features, W, b, transition, tags, mask):
    nc = build()
    in_maps, lens, c_shift, gold = prepare(features, W, b, transition, tags, mask)
    res = bass_utils.run_bass_kernel_spmd(nc, in_maps, core_ids=list(range(NCORES)))
    return finish(res.results, lens, c_shift, gold)
